# revision 71
# baseline (speedup 1.0000x reference)
"""Multi-head causal attention (B=2, T=2048, D=1024, H=16) on 8 TRN2
NeuronCores: data parallel over batch x tensor parallel over head groups
(4 heads per core). Each core computes its group's Q/K/V projections,
causal attention, and a partial output projection; the host sums the 4
partials per batch element.

Self-contained: builds the Bass/Tile kernel, runs it via
run_bass_kernel_spmd on cores 0-7, gathers on host.
"""
import numpy as np

import concourse.bass as bass
import concourse.mybir as mybir
import concourse.tile as tile
from concourse.bass_utils import run_bass_kernel_spmd
from concourse.masks import make_identity

P = 128
B, T, D = 2, 2048, 1024
H_LOCAL = 4          # heads per core
HD = 64              # head dim
F = H_LOCAL * HD     # 256 features per group
KO = D // P          # 8 contraction subtiles
NT = 512             # matmul moving width / PSUM bank
QJ = T // NT         # 4 q column tiles
KT = T // P          # 16 k row tiles
N_CORES = 8
LAG = 3              # S-matmul lookahead over P@V accumulation
LAG2 = 8             # division PE-broadcast deferral (covers ACT recip chain)

f32 = mybir.dt.float32
f16 = mybir.dt.float16

_uid = [0]


def _legalize_single_wait(nc):
    # This walrus build accepts only ONE sem wait per instruction; hoist
    # extra waits onto single-wait NoOps placed just before the instruction.
    for fn in nc.m.functions:
        for bb in fn.blocks:
            new_list = []
            changed = False
            for inst in bb.instructions:
                si = inst.sync_info
                if si is not None and len(si.on_wait) > 1:
                    waits = list(si.on_wait)
                    for w in waits[:-1]:
                        _uid[0] += 1
                        new_list.append(mybir.InstNoOp(
                            name=f"I-waitsplit-{_uid[0]}",
                            engine=inst.engine,
                            sync_info=mybir.SyncInfo(on_wait=[w], on_update=[]),
                        ))
                    inst.sync_info = mybir.SyncInfo(
                        on_wait=[waits[-1]], on_update=list(si.on_update))
                    changed = True
                new_list.append(inst)
            if changed:
                bb.instructions.clear()
                bb.instructions.extend(new_list)


def build_nc():
    nc = bass.Bass(trn_type="TRN2", target_bir_lowering=False, debug=False,
                   num_devices=N_CORES)
    xT = nc.dram_tensor("xT", [D, T], f16, kind="ExternalInput").ap()
    WqT = nc.dram_tensor("WqT", [D, F], f16, kind="ExternalInput").ap()
    WkT = nc.dram_tensor("WkT", [D, F], f16, kind="ExternalInput").ap()
    WvT = nc.dram_tensor("WvT", [D, F], f16, kind="ExternalInput").ap()
    WoT = nc.dram_tensor("WoT", [F, D], f16, kind="ExternalInput").ap()
    TRI = nc.dram_tensor("TRI", [P, P], f16, kind="ExternalInput").ap()
    # f16 partials (summed across 4 cores on host in f32): halves the Z
    # write-back traffic that dominates the kernel tail
    Z = nc.dram_tensor("Z", [T, D], f16, kind="ExternalOutput").ap()

    xTr = xT.rearrange("(ko p) t -> p ko t", p=P)
    w_r = {
        "q": WqT.rearrange("(ko p) f -> p ko f", p=P),
        "k": WkT.rearrange("(ko p) f -> p ko f", p=P),
        "v": WvT.rearrange("(ko p) f -> p ko f", p=P),
    }

    with tile.TileContext(nc) as tc:
        with (
            tc.tile_pool(name="cw", bufs=1) as cw,
            tc.tile_pool(name="sb1", bufs=1) as sb1,
            tc.tile_pool(name="tp", bufs=4) as tp,
            tc.tile_pool(name="psS", bufs=4, space="PSUM") as psS,
            tc.tile_pool(name="psO", bufs=2, space="PSUM") as psO,
            tc.tile_pool(name="psM", bufs=2, space="PSUM") as psM,
        ):
            # ---- persistent constants / staging ----
            w_sb = {}
            for name in ("q", "k", "v"):
                w_sb[name] = sb1.tile([P, KO, F], f16, tag=f"w{name}",
                                      name=f"w{name}")
            xt = sb1.tile([P, KO, T], f16, tag="xt", name="xt")
            # each dma_start costs ~600ns on the serialized SP sequencer
            # while the transfer itself sprays across all 16 queues at
            # aggregate BW - so issue ONE whole-tensor trigger per tensor,
            # earliest-needed first (wq+xt@qj0 gate the 1st matmul)
            nc.sync.dma_start(xt[:, :, 0:NT // 2], xTr[:, :, 0:NT // 2])
            nc.sync.dma_start(w_sb["q"][:], w_r["q"][:])
            nc.sync.dma_start(xt[:, :, NT // 2:NT], xTr[:, :, NT // 2:NT])
            nc.sync.dma_start(w_sb["k"][:], w_r["k"][:])
            nc.sync.dma_start(w_sb["v"][:], w_r["v"][:])
            nc.sync.dma_start(xt[:, :, NT:T], xTr[:, :, NT:T])

            wo = cw.tile([P, F // P, D], f16, tag="wo", name="wo")
            tri = cw.tile([P, P], f16, tag="tri", name="tri")
            ident = cw.tile([P, P], f16, tag="ident", name="ident")
            make_identity(nc, ident[:])

            # Q packed 2 heads per 128-row subtile (natural psum layout).
            # K zero-padded per head: subtile h holds head h's 64 features at
            # partition rows 64*(h%2)..+64 (matching Q's rows), zeros
            # elsewhere - so K=128 S-matmuls see only head h.
            qt = cw.tile([P, F // P, T], f16, tag="qt", name="qt")
            ktz = cw.tile([P, H_LOCAL, T], f16, tag="ktz", name="ktz")
            nc.gpsimd.memset(ktz[:], 0.0)

            # V with a ones column per head: [k-token, kt, head, 0:64]=V^T,
            # [..., 64]=1 (gives softmax denominators for free in P@V)
            vaug = cw.tile([P, KT, H_LOCAL, HD + 1], f16, tag="vaug", name="vaug")
            nc.gpsimd.memset(vaug[:, :, :, HD:HD + 1], 1.0)
            # ones rows at partitions 0 and 64: PE operands must share their
            # base partition, and the pair's two reciprocal rows live at 0/64
            ones_r = cw.tile([HD + 1, HD], f16, tag="ones", name="ones")
            nc.gpsimd.memset(ones_r[0:1, :], 1.0)
            nc.gpsimd.memset(ones_r[HD:HD + 1, :], 1.0)


            ot = cw.tile([P, F // P, T], f16, tag="ot", name="ot")

            # gpsimd-DGE loads issued after the memsets above: tri isn't
            # needed before the first diagonal S block (~25us) and wo not
            # before the first spliced phase4 (~90us), so keep their traffic
            # out of the critical first xt/wq window
            nc.gpsimd.dma_start(tri[:], TRI)
            nc.gpsimd.dma_start(wo[:], WoT.rearrange("(fo p) d -> p fo d", p=P))

            def phase1_chunks(qj):
                # emission chunks (each ~8 PE matmuls) to splice between
                # attention heads so the PE stream never drains
                sl = slice(qj * NT, (qj + 1) * NT)
                vt = tp.tile([P, F // P, NT], f16, tag="vt", bufs=2,
                             name=f"vt{qj}")
                chunks = []

                def proj(name, fs):
                    def emit():
                        ps = psS.tile([P, NT], f32, tag="s",
                                      name=f"ps_{name}{fs}_{qj}")
                        if qj == 0 and name == "q":
                            # warmup: x@qj0 arrives as two half-column DMAs;
                            # column-split chains start on the first half
                            for c0 in (0, NT // 2):
                                for ko in range(KO):
                                    nc.tensor.matmul(
                                        ps[:, c0:c0 + NT // 2],
                                        w_sb[name][:, ko, fs * P:(fs + 1) * P],
                                        xt[:, ko, c0:c0 + NT // 2],
                                        start=(ko == 0), stop=(ko == KO - 1))
                            # fall through to the copies below
                        else:
                            for ko in range(KO):
                                nc.tensor.matmul(
                                    ps[:], w_sb[name][:, ko, fs * P:(fs + 1) * P],
                                    xt[:, ko, sl],
                                    start=(ko == 0), stop=(ko == KO - 1))
                        if name == "q":
                            nc.vector.tensor_copy(qt[:, fs, sl], ps[:])
                        elif name == "k":
                            nc.vector.tensor_copy(ktz[0:HD, 2 * fs, sl], ps[0:HD])
                            nc.vector.tensor_copy(ktz[HD:P, 2 * fs + 1, sl], ps[HD:P])
                        else:
                            nc.vector.tensor_copy(vt[:, fs, :], ps[:])
                    return emit

                def vtrans(fs):
                    def emit():
                        for kt in range(4 * qj, 4 * qj + 4):
                            loc = kt - 4 * qj
                            pst = psM.tile([P, P], f16, tag="m",
                                           name=f"pvt{fs}_{kt}")
                            nc.tensor.transpose(
                                pst[:], vt[:, fs, loc * P:(loc + 1) * P], ident[:])
                            nc.vector.tensor_copy(
                                vaug[:, kt, 2 * fs:2 * fs + 2, 0:HD],
                                pst.rearrange("p (a b) -> p a b", a=2))
                    return emit

                for name in ("q", "k", "v"):
                    for fs in range(F // P):
                        chunks.append(proj(name, fs))
                for fs in range(F // P):
                    chunks.append(vtrans(fs))
                return chunks

            def phase23(h, qj, pair, pending=None):
                po = psO.tile([HD + 1, NT], f32, tag="o", name=f"po{h}_{qj}")
                n_ki = 4 * qj + 4
                pts = {}

                def s_step(ki):
                    col0 = 0 if ki < 4 * qj else (ki - 4 * qj) * P
                    N = NT - col0
                    ps = psS.tile([P, NT], f32, tag="s", name=f"pss{h}_{qj}_{ki}")
                    nc.tensor.matmul(
                        ps[:, 0:N], ktz[:, h, ki * P:(ki + 1) * P],
                        qt[:, h // 2, qj * NT + col0:(qj + 1) * NT],
                        start=True, stop=True)
                    pt = tp.tile([P, NT], f16, tag="pt", bufs=6,
                                 name=f"pt{h}_{qj}_{ki}")
                    nc.scalar.activation(pt[:, 0:N], ps[:, 0:N],
                                         mybir.ActivationFunctionType.Exp,
                                         scale=0.125)
                    if ki >= 4 * qj:
                        nc.vector.tensor_mul(pt[:, 0:P], pt[:, 0:P], tri[:])
                    pts[ki] = (pt, col0, N)

                def o_step(ki):
                    pt, col0, N = pts.pop(ki)
                    nc.tensor.matmul(
                        po[:, col0:NT], vaug[:, ki, h, :], pt[:, 0:N],
                        start=(ki == 0), stop=(ki == n_ki - 1))

                for ki in range(n_ki + LAG):
                    if ki < n_ki:
                        s_step(ki)
                    # previous head's deferred division apply: by now its
                    # ACT recip chain (emitted at the end of that head,
                    # drained during the splice) is long finished, so the
                    # PE broadcast never heads the PE stream waiting on it
                    if ki == LAG2 and pending is not None:
                        pending()
                        pending = None
                    if ki >= LAG:
                        o_step(ki - LAG)
                if pending is not None:
                    pending()

                # stage this head's numerators and denominator row into the
                # PAIR tiles (releases po). Denominators sit at partition
                # rows 0 and 64 (the only legal sub-tile offsets); the
                # reciprocal is batched per head pair: one Ln + one Exp over
                # the [65, NT] tile cost the same as [1, NT] on ACT (free-dim
                # priced), halving the division's ACT displacement of the
                # softmax exp cadence. Rows 1..63 hold garbage - unused.
                j = h % 2
                so2, den, lnt, rrt = pair
                r0 = HD * j
                nc.vector.tensor_copy(so2[r0:r0 + HD, :], po[0:HD, :])

                if qj == QJ - 1 and h >= 2:
                    # last pair runs solo so the final division chain (which
                    # gates the tail phase4) is as short as possible: ln
                    # reads the PSUM den row directly, no staging copy
                    nc.scalar.activation(lnt[r0:r0 + 1, :], po[HD:HD + 1, :],
                                         mybir.ActivationFunctionType.Ln)
                    nc.scalar.activation(rrt[r0:r0 + 1, :], lnt[r0:r0 + 1, :],
                                         mybir.ActivationFunctionType.Exp,
                                         scale=-1.0)

                    def div_apply_solo():
                        pb = psM.tile([P, NT], f32, tag="m",
                                      name=f"pb{h}_{qj}")
                        nc.tensor.matmul(pb[r0:r0 + HD],
                                         ones_r[r0:r0 + 1, :],
                                         rrt[r0:r0 + 1, :],
                                         start=True, stop=True)
                        nc.vector.tensor_mul(
                            ot[r0:r0 + HD, h // 2, qj * NT:(qj + 1) * NT],
                            so2[r0:r0 + HD, :], pb[r0:r0 + HD])
                    # the incoming pending (h0/h1 pair apply) was already
                    # emitted at ki==LAG2 inside the loop above
                    return div_apply_solo

                nc.vector.tensor_copy(den[r0:r0 + 1, :], po[HD:HD + 1, :])
                if j == 0:
                    return pending

                nc.scalar.activation(lnt[:], den[:],
                                     mybir.ActivationFunctionType.Ln)
                nc.scalar.activation(rrt[:], lnt[:],
                                     mybir.ActivationFunctionType.Exp,
                                     scale=-1.0)

                def div_apply():
                    # broadcast both reciprocal rows, one multiply per pair
                    pb = psM.tile([P, NT], f32, tag="m", name=f"pb{h}_{qj}")
                    nc.tensor.matmul(pb[0:HD], ones_r[0:1, :], rrt[0:1, :],
                                     start=True, stop=True)
                    nc.tensor.matmul(pb[HD:P], ones_r[HD:HD + 1, :],
                                     rrt[HD:HD + 1, :], start=True, stop=True)
                    nc.vector.tensor_mul(
                        ot[:, h // 2, qj * NT:(qj + 1) * NT], so2[:], pb[:])
                return div_apply

            def phase4(qt, tail=False):
                # tail calls run after all attention: psS (4 bufs) is idle
                # then and gives a deeper pz pipeline than psM (2 bufs)
                pool, tag = (psS, "s") if tail else (psM, "m")
                zs = tp.tile([P, D], f16, tag="z", bufs=2, name=f"zs{qt}")
                for dt in range(D // NT):
                    pz = pool.tile([P, NT], f32, tag=tag, name=f"pz{qt}_{dt}")
                    for fs in range(F // P):
                        nc.tensor.matmul(
                            pz[:], ot[:, fs, qt * P:(qt + 1) * P],
                            wo[:, fs, dt * NT:(dt + 1) * NT],
                            start=(fs == 0), stop=(fs == F // P - 1))
                    # DVE only: ACT is the local bottleneck while phase4 is
                    # spliced between attention heads (softmax exps + recip)
                    nc.vector.tensor_copy(zs[:, dt * NT:(dt + 1) * NT], pz[:])
                # one whole-row DMA trigger per qt (SP sequencer is the
                # scarce resource, the transfer itself sprays all queues)
                nc.sync.dma_start(Z[qt * P:(qt + 1) * P, :], zs[:])

            pending = None
            for c in phase1_chunks(0):
                c()
            p4_backlog = []
            hold = []
            for qj in range(QJ):
                splice = list(phase1_chunks(qj + 1)) if qj + 1 < QJ else []
                if qj == 2:
                    splice += p4_backlog[:4]       # phase4 of qj 0
                    p4_backlog = p4_backlog[4:]
                elif qj == 3:
                    # phase4 of qj 1 and 2; keep two back to keep the PE fed
                    # during the last head's division latency
                    splice += p4_backlog[:6]
                    hold = p4_backlog[6:]
                    p4_backlog = []
                n_h = H_LOCAL
                pair = None
                for h in range(n_h):
                    if h % 2 == 0:
                        pair = (
                            tp.tile([P, NT], f32, tag="so", bufs=2,
                                    name=f"so{qj}_{h // 2}"),
                            tp.tile([HD + 1, NT], f32, tag="dn", bufs=2,
                                    name=f"dn{qj}_{h // 2}"),
                            tp.tile([HD + 1, NT], f32, tag="lnt", bufs=2,
                                    name=f"lnt{qj}_{h // 2}"),
                            tp.tile([HD + 1, NT], f16, tag="rr", bufs=2,
                                    name=f"rr{qj}_{h // 2}"),
                        )

                    pending = phase23(h, qj, pair, pending)
                    k0 = (len(splice) * h) // n_h
                    k1 = (len(splice) * (h + 1)) // n_h
                    for c in splice[k0:k1]:
                        c()
                p4_backlog += [(lambda qt=qt, **kw: phase4(qt, **kw))
                               for qt in range(4 * qj, 4 * qj + 4)]
            for c in hold:
                c(tail=True)
            if pending is not None:
                pending()
            for c in p4_backlog:
                c(tail=True)

    _legalize_single_wait(nc)
    return nc


_TRI = None


def _make_in_maps(x, Wq, Wk, Wv, Wo):
    global _TRI
    if _TRI is None:
        # allowed[k_row, q_col] = q >= k  (upper-triangular incl. diagonal)
        _TRI = (np.arange(P)[None, :] >= np.arange(P)[:, None]).astype(np.float16)
    in_maps = []
    for c in range(N_CORES):
        b, g = divmod(c, 4)
        sl = slice(g * F, (g + 1) * F)
        in_maps.append({
            "xT": np.ascontiguousarray(np.asarray(x)[b].T.astype(np.float16)),
            "WqT": np.ascontiguousarray(np.asarray(Wq)[sl, :].T.astype(np.float16)),
            "WkT": np.ascontiguousarray(np.asarray(Wk)[sl, :].T.astype(np.float16)),
            "WvT": np.ascontiguousarray(np.asarray(Wv)[sl, :].T.astype(np.float16)),
            "WoT": np.ascontiguousarray(np.asarray(Wo)[:, sl].T.astype(np.float16)),
            "TRI": _TRI,
        })
    return in_maps


def run(x, Wq, Wk, Wv, Wo, trace=False, trace_cores=None):
    nc = build_nc()
    in_maps = _make_in_maps(x, Wq, Wk, Wv, Wo)
    res = run_bass_kernel_spmd(nc, in_maps, list(range(N_CORES)), trace=trace,
                               trace_cores=trace_cores)
    out = np.zeros((B, T, D), np.float32)
    for c in range(N_CORES):
        out[c // 4] += res.results[c]["Z"].astype(np.float32)
    return out, res


def kernel(x, Wq, Wk, Wv, Wo):
    try:
        out, _ = run(x, Wq, Wk, Wv, Wo)
    except Exception:
        # one retry for transient device errors (e.g. a wedged core from a
        # prior run)
        out, _ = run(x, Wq, Wk, Wv, Wo)
    return out

